# revision 2
# baseline (speedup 1.0000x reference)
"""GCN 2-layer + link decode on 8 TRN2 NeuronCores (full inputs in/out).

Design (dest-sharded, scatter-free):
- Aggregation commutes with the weight matmul: h = relu(segsum(w1*x[src]) @ W1);
  decode tables [u|v] = segsum2(w2*h[src]) @ (W2 @ [WlinA.T|WlinB.T]).
- Each core owns 12544 destination slots. Edge streams sorted by
  (src int16-range, dest chunk, dest); bulk-gathered via dma_gather (int16
  local indices per 32768-row range); routed+weighted into per-chunk PSUM by
  a selection-matrix matmul whose lhsT is built in ONE fused DVE op:
  sel[e,m] = (iota[e,m] == rel_slot[e]) * w[e].
- Per-(chunk,range) cell sizes are max-padded across cores so the schedule is
  SPMD-uniform; per-core routing differences live in the rel/w data.
- h (100352x128 f32) is AllGather'd between layers; uv (4 f32/node padded to
  64 cols for the 256B dma_gather row constraint) AllGather'd before decode.
- Decode: pairs sorted by pos0, sharded by slot windows; u and v streams both
  sel-routed into pair-chunk PSUM; host unshuffles the output rows.
"""
import numpy as np

P = 128
N = 100_000
NSHARD = 12_500
SLOTS = 12_544
CHUNKS = SLOTS // P          # 98
TABROWS = 8 * SLOTS          # 100352
RBOUND = [32768, 65536, 98304]
RLO = [0, 32768, 65536, 98304]
NCORES = 8
CALL_CELLS = 8               # chunks per gather-call window


def _range_of(a):
    return np.searchsorted(RBOUND, a, side="right")


def _wrap_idx(a):
    """[NCORES, T] int16 -> [NCORES, 128, T//16] (16-wrap, 8x replicate)."""
    ncr, t = a.shape
    out = a.reshape(ncr, t // 16, 16).transpose(0, 2, 1)
    return np.ascontiguousarray(np.tile(out, (1, 8, 1)))


def _prep_stream(tab_row, slot, w, nchunks, call_cells):
    """Generic SPMD-uniform stream builder.

    tab_row: [E] global table row per entry; slot: [E] local out slot
    (0..nchunks*128); w: [E] weight; entries already per-core-filtered lists:
    tab_row etc are lists of arrays, one per core.
    Returns static schedule + per-core idx16 / rel / w arrays.
    """
    ncr = len(tab_row)
    # cell = (chunk, range); count per core
    counts = np.zeros((ncr, nchunks, 4), np.int64)
    for c in range(ncr):
        ch = slot[c] // P
        rr = _range_of(tab_row[c])
        np.add.at(counts, (c, ch, rr), 1)
    estar = counts.max(axis=0)                       # [nchunks, 4]

    # layout per range: calls of CALL windows, each padded to 128 multiple
    layout = []
    for r in range(4):
        calls = []
        base = 0
        for k0 in range(0, nchunks, call_cells):
            k1 = min(k0 + call_cells, nchunks)
            cells = estar[k0:k1, r]
            offs = np.concatenate([[0], np.cumsum(cells)]).astype(np.int64)
            n = int(offs[-1])
            n_pad = max(P, ((n + P - 1) // P) * P)
            calls.append(dict(k0=k0, k1=k1, offs=offs, n=n, n_pad=n_pad,
                              base=base))
            base += n_pad
        layout.append(dict(calls=calls, T=base))

    # static schedule: per chunk, matmul descriptors (r, call, blk, sel_col)
    sched = [[] for _ in range(nchunks)]
    selmap = {}
    n_sel = 0
    for r in range(4):
        for ci, call in enumerate(layout[r]["calls"]):
            nblk = call["n_pad"] // P
            offs, k0 = call["offs"], call["k0"]
            for b in range(nblk):
                e0, e1 = b * P, b * P + P
                ks = [k for k in range(call["k0"], call["k1"])
                      if offs[k - k0] < e1 and offs[k - k0 + 1] > e0]
                if not ks:
                    ks = [call["k0"]]
                for k in ks:
                    sched[k].append(dict(r=r, call=ci, blk=b, sel=n_sel))
                    selmap[(r, ci, b, k)] = n_sel
                    n_sel += 1

    idx16 = [np.zeros((ncr, layout[r]["T"]), np.int16) for r in range(4)]
    rel = np.zeros((ncr, P, n_sel), np.float32)
    wgt = np.zeros((ncr, P, n_sel), np.float32)

    for c in range(ncr):
        tr, sl, ww = tab_row[c], slot[c], w[c]
        rr = _range_of(tr)
        ch = sl // P
        # order entries by (range, chunk, slot)
        o = np.lexsort((sl, ch, rr))
        tr, sl, ww, rr, ch = tr[o], sl[o], ww[o], rr[o], ch[o]
        for r in range(4):
            m = rr == r
            if not m.any():
                continue
            trm, slm, wwm, chm = tr[m], sl[m], ww[m], ch[m]
            # position: call base + cell offset + within-cell rank
            cell_cnt = np.zeros(nchunks, np.int64)
            np.add.at(cell_cnt, chm, 1)
            cstart = np.concatenate([[0], np.cumsum(cell_cnt)])
            within = np.arange(len(slm)) - cstart[chm]
            call_id = chm // call_cells
            calls = layout[r]["calls"]
            cbase = np.array([cl["base"] for cl in calls], np.int64)
            # offset of chunk's cell within its call
            cell_off = np.zeros(nchunks, np.int64)
            for ci, cl in enumerate(calls):
                for k in range(cl["k0"], cl["k1"]):
                    cell_off[k] = cl["offs"][k - cl["k0"]]
            pos = cbase[call_id] + cell_off[chm] + within
            idx16[r][c, pos] = (trm - RLO[r]).astype(np.int16)
            # sel column per entry
            relpos = pos - cbase[call_id]
            blk = relpos // P
            pp = relpos % P
            cols = np.array([selmap[(r, int(ci_), int(b_), int(k_))]
                             for ci_, b_, k_ in zip(call_id, blk, chm)],
                            np.int64)
            rel[c, pp, cols] = (slm % P).astype(np.float32)
            wgt[c, pp, cols] = wwm
    return dict(layout=layout, sched=sched, n_sel=n_sel, idx16=idx16,
                rel=rel, wgt=wgt)


def kernel(x, edge_index1, edge_index2, edge_weight1, edge_weight2,
           pos_edge_index, W1, W2, Wlin):
    import concourse.bass as bass
    from concourse import bacc, tile, mybir
    from concourse.bass_utils import run_bass_kernel_spmd
    from concourse.library_config import mlp
    from concourse.masks import make_identity

    f32, i16 = mybir.dt.float32, mybir.dt.int16
    x = np.asarray(x, np.float32)
    W1 = np.asarray(W1, np.float32)
    W2 = np.asarray(W2, np.float32)
    Wlin = np.asarray(Wlin, np.float32)
    e1 = np.asarray(edge_index1).astype(np.int64)
    e2 = np.asarray(edge_index2).astype(np.int64)
    w1 = np.asarray(edge_weight1, np.float32)
    w2 = np.asarray(edge_weight2, np.float32)
    pe = np.asarray(pos_edge_index).astype(np.int64)

    # ---------- host index preprocessing ----------
    x_tab = np.zeros((TABROWS, P), np.float32)
    x_tab[:N] = x
    n2row = (np.arange(N) // NSHARD) * SLOTS + (np.arange(N) % NSHARD)

    def shard_by_dest(src_rows, dst, w):
        owner = dst // NSHARD
        ld = dst - owner * NSHARD
        return ([src_rows[owner == c] for c in range(NCORES)],
                [ld[owner == c] for c in range(NCORES)],
                [w[owner == c] for c in range(NCORES)])

    l1 = _prep_stream(*shard_by_dest(e1[0], e1[1], w1), CHUNKS, CALL_CELLS)
    l2 = _prep_stream(*shard_by_dest(n2row[e2[0]], e2[1], w2),
                      CHUNKS, CALL_CELLS)

    # decode: shard pairs by original index; per core sort by pos0-row and
    # use the local sorted position as the output slot (host unshuffles).
    t0 = n2row[pe[0]]
    t1 = n2row[pe[1]]
    npairs = pe.shape[1]
    pershard = (npairs + NCORES - 1) // NCORES            # 25000
    per_core = ((pershard + P - 1) // P) * P              # 25088
    dchunks = per_core // P
    pair_slot = np.empty(npairs, np.int64)
    u_rows, u_slots, u_w = [], [], []
    v_rows, v_slots, v_w = [], [], []
    for c in range(NCORES):
        p0, p1 = c * pershard, min((c + 1) * pershard, npairs)
        loc = np.argsort(t0[p0:p1], kind="stable")
        sl = np.empty(p1 - p0, np.int64)
        sl[loc] = np.arange(p1 - p0)
        pair_slot[p0:p1] = c * per_core + sl
        ones = np.ones(p1 - p0, np.float32)
        u_rows.append(t0[p0:p1]); u_slots.append(sl); u_w.append(ones)
        v_rows.append(t1[p0:p1]); v_slots.append(sl); v_w.append(ones)
    du = _prep_stream(u_rows, u_slots, u_w, dchunks, 48)
    dv = _prep_stream(v_rows, v_slots, v_w, dchunks, 48)

    idx_arr = {}
    for key, pr in (("l1", l1), ("l2", l2), ("u", du), ("v", dv)):
        for r in range(4):
            idx_arr[(key, r)] = _wrap_idx(pr["idx16"][r])

    # ---------- device program ----------
    nc = bacc.Bacc("TRN2", target_bir_lowering=False, debug=False,
                   num_devices=NCORES, num_swdge_queues=4)

    def din(name, shape, dt=f32):
        return nc.dram_tensor(name, list(shape), dt, kind="ExternalInput").ap()

    xt = din("x_tab", (TABROWS, P))
    w1t = din("W1r", (P, P))
    w2tt = din("W2T", (P, P))
    wcat = din("Wcat", (P, 4))
    iota_in = din("iota", (P, P))
    idx_in = {k: din(f"idx_{k[0]}_{k[1]}", v.shape[1:], i16)
              for k, v in idx_arr.items()}
    relw_in = {key: (din(f"rel_{key}", (P, pr["n_sel"])),
                     din(f"w_{key}", (P, pr["n_sel"])))
               for key, pr in (("l1", l1), ("l2", l2), ("u", du), ("v", dv))}

    out_d = nc.dram_tensor("out_dec", [P, 2 * dchunks], f32,
                           kind="ExternalOutput").ap()
    h_slice = nc.dram_tensor("h_slice", [SLOTS, P], f32)
    h_tab = nc.dram_tensor("h_tab", [TABROWS, P], f32, addr_space="Shared")
    uv_slice = nc.dram_tensor("uv_slice", [SLOTS, 64], f32)
    uv_tab = nc.dram_tensor("uv_tab", [TABROWS, 64], f32, addr_space="Shared")

    qn = [0]

    def next_q():
        qn[0] = (qn[0] + 1) % 4
        return qn[0]

    with tile.TileContext(nc) as tc:
        with (
            tc.tile_pool(name="meta", bufs=1) as mp,
            tc.tile_pool(name="stage", bufs=2) as sgp,
            tc.tile_pool(name="idxp", bufs=1) as ixp,
            tc.tile_pool(name="selp", bufs=4) as selp,
            tc.tile_pool(name="work", bufs=3) as wp,
            tc.tile_pool(name="psA", bufs=2, space="PSUM") as ppA,
            tc.tile_pool(name="psB", bufs=2, space="PSUM") as ppB,
        ):
            nc.gpsimd.load_library(mlp)
            iota_t = mp.tile([P, P], f32, name="iota_t")
            nc.sync.dma_start(iota_t[:], iota_in[:])
            ident = mp.tile([P, P], f32, name="ident")
            make_identity(nc, ident[:])
            w1_sb = mp.tile([P, P], f32, name="w1_sb")
            nc.sync.dma_start(w1_sb[:], w1t[:])
            w2t_sb = mp.tile([P, P], f32, name="w2t_sb")
            nc.sync.dma_start(w2t_sb[:], w2tt[:])
            wcat_sb = mp.tile([P, 4], f32, name="wcat_sb")
            nc.sync.dma_start(wcat_sb[:], wcat[:])
            wu_ps = ppB.tile([P, 4], f32, space="PSUM", name="wu_ps",
                             tag="psB")
            nc.tensor.matmul(wu_ps[:], lhsT=w2t_sb[:], rhs=wcat_sb[:],
                             start=True, stop=True)
            wu_sb = mp.tile([P, 4], f32, name="wu_sb")
            nc.vector.tensor_copy(wu_sb[:], wu_ps[:])

            def sel_build(name, rel_sb, w_sb, col):
                sel = selp.tile([P, P], f32, name=name, tag="sel")
                nc.vector.scalar_tensor_tensor(
                    out=sel[:], in0=iota_t[:],
                    scalar=rel_sb[:, col:col + 1],
                    in1=w_sb[:, col:col + 1].to_broadcast([P, P]),
                    op0=mybir.AluOpType.is_equal,
                    op1=mybir.AluOpType.mult)
                return sel

            def load_relw(key, pr):
                rel_sb = ixp.tile([P, pr["n_sel"]], f32,
                                  name=f"rel_{key}_sb", tag="relt")
                w_sb = ixp.tile([P, pr["n_sel"]], f32,
                                name=f"w_{key}_sb", tag="wt")
                nc.sync.dma_start(rel_sb[:], relw_in[key][0][:])
                nc.sync.dma_start(w_sb[:], relw_in[key][1][:])
                return rel_sb, w_sb

            def run_agg(key, pr, tab_ap, nchunks, elem, consume):
                """Gather + sel-route; call consume(k, psum_tile) per chunk."""
                rel_sb, w_sb = load_relw(key, pr)
                idx_sb = []
                for r in range(4):
                    cols = pr["layout"][r]["T"] // 16
                    it = ixp.tile([P, cols], i16, name=f"ix_{key}_{r}",
                                  tag=f"ix{r}")
                    nc.sync.dma_start(it[:], idx_in[(key, r)][:])
                    idx_sb.append(it)
                stage_tiles = {}

                def ensure_call(r, ci):
                    if (r, ci) in stage_tiles:
                        return stage_tiles[(r, ci)]
                    call = pr["layout"][r]["calls"][ci]
                    npad = call["n_pad"]
                    c0 = call["base"] // 16
                    st = sgp.tile([P, (npad // P) * elem], f32,
                                  name=f"st_{key}_{r}_{ci}", tag=f"stage{r}")
                    nc.gpsimd.dma_gather(
                        st[:].rearrange("p (c e) -> p c e", e=elem),
                        tab_ap[RLO[r]:], idx_sb[r][:, c0:c0 + npad // 16],
                        npad, npad, elem,
                        queue_num=next_q(), single_packet=False)
                    stage_tiles[(r, ci)] = st
                    return st

                for k in range(nchunks):
                    psum_k = ppA.tile([P, elem], f32, space="PSUM",
                                      name=f"ps_{key}_{k}", tag="psA")
                    descs = pr["sched"][k]
                    for j, d in enumerate(descs):
                        st = ensure_call(d["r"], d["call"])
                        sel = sel_build(f"sel_{key}_{k}_{j}", rel_sb, w_sb,
                                        d["sel"])
                        nc.tensor.matmul(
                            psum_k[:], lhsT=sel[:],
                            rhs=st[:, d["blk"] * elem:(d["blk"] + 1) * elem],
                            start=(j == 0), stop=(j == len(descs) - 1))
                    consume(k, psum_k)

            def consume_l1(k, psum_k):
                a_sb = wp.tile([P, P], f32, name=f"a1_{k}", tag="a")
                nc.vector.tensor_copy(a_sb[:], psum_k[:])
                at_ps = ppB.tile([P, P], f32, space="PSUM",
                                 name=f"at1_{k}", tag="psB")
                nc.tensor.transpose(at_ps[:], a_sb[:], ident[:])
                at_sb = wp.tile([P, P], f32, name=f"at1s_{k}", tag="at")
                nc.vector.tensor_copy(at_sb[:], at_ps[:])
                h_ps = ppB.tile([P, P], f32, space="PSUM",
                                name=f"h1_{k}", tag="psB")
                nc.tensor.matmul(h_ps[:], lhsT=at_sb[:], rhs=w1_sb[:],
                                 start=True, stop=True)
                h_sb = wp.tile([P, P], f32, name=f"h1s_{k}", tag="h")
                nc.vector.tensor_scalar_max(h_sb[:], h_ps[:], 0.0)
                nc.sync.dma_start(h_slice[k * P:(k + 1) * P, :], h_sb[:])

            def consume_l2(k, psum_k):
                a_sb = wp.tile([P, P], f32, name=f"a2_{k}", tag="a")
                nc.vector.tensor_copy(a_sb[:], psum_k[:])
                at_ps = ppB.tile([P, P], f32, space="PSUM",
                                 name=f"at2_{k}", tag="psB")
                nc.tensor.transpose(at_ps[:], a_sb[:], ident[:])
                at_sb = wp.tile([P, P], f32, name=f"at2s_{k}", tag="at")
                nc.vector.tensor_copy(at_sb[:], at_ps[:])
                u_ps = ppB.tile([P, 4], f32, space="PSUM",
                                name=f"uv_{k}", tag="psB")
                nc.tensor.matmul(u_ps[:], lhsT=at_sb[:], rhs=wu_sb[:],
                                 start=True, stop=True)
                u_sb = wp.tile([P, 64], f32, name=f"uvs_{k}", tag="u")
                nc.vector.tensor_copy(u_sb[:, 0:4], u_ps[:])
                nc.sync.dma_start(uv_slice[k * P:(k + 1) * P, :], u_sb[:])

            run_agg("l1", l1, xt, CHUNKS, P, consume_l1)
            nc.gpsimd.collective_compute(
                "AllGather", mybir.AluOpType.bypass,
                replica_groups=[list(range(NCORES))],
                ins=[h_slice[:]], outs=[h_tab[:]])
            run_agg("l2", l2, h_tab[:], CHUNKS, P, consume_l2)
            nc.gpsimd.collective_compute(
                "AllGather", mybir.AluOpType.bypass,
                replica_groups=[list(range(NCORES))],
                ins=[uv_slice[:]], outs=[uv_tab[:]])

            # decode: u then v aggregated into [pairs, 64] psums; add
            out_sb = mp.tile([P, 2 * dchunks], f32, name="out_sb")
            u_all = mp.tile([P, 4 * dchunks], f32, name="u_all")

            def consume_u(k, psum_k):
                nc.vector.tensor_copy(u_all[:, 4 * k:4 * k + 4],
                                      psum_k[:, 0:4])

            def consume_v(k, psum_k):
                nc.vector.tensor_tensor(
                    out=out_sb[:, 2 * k:2 * k + 2],
                    in0=u_all[:, 4 * k:4 * k + 2], in1=psum_k[:, 2:4],
                    op=mybir.AluOpType.add)

            run_agg("u", du, uv_tab[:], dchunks, 64, consume_u)
            run_agg("v", dv, uv_tab[:], dchunks, 64, consume_v)
            nc.sync.dma_start(out_d[:], out_sb[:])

    nc.compile()

    # ---------- stage inputs & run ----------
    iota_np = np.broadcast_to(np.arange(P, dtype=np.float32)[None, :],
                              (P, P)).copy()
    wcat_np = np.ascontiguousarray(
        np.concatenate([Wlin[:, :P].T, Wlin[:, P:].T], axis=1))
    in_maps = []
    for c in range(NCORES):
        m = {"x_tab": x_tab, "W1r": W1,
             "W2T": np.ascontiguousarray(W2.T), "Wcat": wcat_np,
             "iota": iota_np}
        for key, pr in (("l1", l1), ("l2", l2), ("u", du), ("v", dv)):
            m[f"rel_{key}"] = np.ascontiguousarray(pr["rel"][c])
            m[f"w_{key}"] = np.ascontiguousarray(pr["wgt"][c])
            for r in range(4):
                m[f"idx_{key}_{r}"] = idx_arr[(key, r)][c]
        in_maps.append(m)

    res = run_bass_kernel_spmd(nc, in_maps, core_ids=list(range(NCORES)),
                               trace=globals().get("TRACE", False))
    globals()["LAST_EXEC_NS"] = res.exec_time_ns
    globals()["LAST_RES"] = res

    out = np.zeros((npairs, 2), np.float32)
    for c in range(NCORES):
        o3 = res.results[c]["out_dec"].reshape(P, dchunks, 2)
        m = (pair_slot >= c * per_core) & (pair_slot < (c + 1) * per_core)
        sl = pair_slot[m] - c * per_core
        out[m] = o3[sl % P, sl // P]
    return out



# revision 18
# speedup vs baseline: 1.4081x; 1.4081x over previous
"""GCN 2-layer + link decode on 8 TRN2 NeuronCores (full inputs in/out).

v2 design (fp16, transposed routing, local decode):
- Dest-sharded edge parallelism: each core owns 12544 destination slots.
  Aggregation commutes with the weight matmul: h = relu(segsum(w1*x[src]) @ W1).
- fp16 tables/streams halve gather bytes; matmuls fp16 (4x faster than f32).
- Transposed routing: psum_T[c, slot] += st[e, c]^T @ sel[e, slot], where
  sel is a pure batched is_equal (wide DVE op over SELBATCH descs, stride-0
  APs) and the edge weight is folded into the staged rows (wide in-place
  mult). Consume needs no PE transposes: h = (psum_T copy) as lhsT @ W1.
- Layer 2 collapses straight to the 4-wide decode table:
  uv_T[:, slot] = (W2 @ [WlinA.T|WlinB.T])^T-replicated @ agg2_T, stored
  f32 in SBUF replicated to 128 partitions.
- Decode is fully local (pairs sharded by owner of p0 for u, p1 for v):
  ap_gather picks uv columns per pair; host adds u+v parts. No 2nd
  AllGather, no decode matmuls, no decode DMA packets.
- Only collective: AllGather of h (fp16).
"""
import numpy as np

P = 128
N = 100_000
NSHARD = 12_500
SLOTS = 12_544
CHUNKS = SLOTS // P          # 98
TABROWS = 8 * SLOTS          # 100352
RBOUND = [32768, 65536, 98304]
RLO = [0, 32768, 65536, 98304]
NCORES = 8
CALL_CELLS = 12              # chunks per gather-call window
SELBATCH = 32                # sel descs built per wide DVE op
DWIN = 4096                  # decode ap_gather window


def _range_of(a):
    return np.searchsorted(RBOUND, a, side="right")


def _wrap_idx(a):
    """[NCORES, T] int16 -> [NCORES, 128, T//16] (16-wrap, 8x replicate)."""
    ncr, t = a.shape
    out = a.reshape(ncr, t // 16, 16).transpose(0, 2, 1)
    return np.ascontiguousarray(np.tile(out, (1, 8, 1)))


def _prep_stream(tab_row, slot, w, nchunks, call_cells):
    """SPMD-uniform stream builder (dest-sharded edge streams).

    tab_row: per-core list of [E_c] global table rows; slot: local out slot;
    w: weight. Returns layout/schedule + per-core idx16 / rel / w arrays.
    rel[(p, col)] = within-chunk slot for the desc's entries, -1 otherwise
    (weights are folded into staged data, not sel).
    wst[r][c][p, b] = weight of stream entry b*128+p of range r (0 on pads).
    """
    ncr = len(tab_row)
    counts = np.zeros((ncr, nchunks, 4), np.int64)
    for c in range(ncr):
        ch = slot[c] // P
        rr = _range_of(tab_row[c])
        np.add.at(counts, (c, ch, rr), 1)
    estar = counts.max(axis=0)                       # [nchunks, 4]

    layout = []
    for r in range(4):
        calls = []
        base = 0
        for k0 in range(0, nchunks, call_cells):
            k1 = min(k0 + call_cells, nchunks)
            cells = estar[k0:k1, r]
            offs = np.concatenate([[0], np.cumsum(cells)]).astype(np.int64)
            n = int(offs[-1])
            n_pad = max(P, ((n + P - 1) // P) * P)
            calls.append(dict(k0=k0, k1=k1, offs=offs, n=n, n_pad=n_pad,
                              base=base))
            base += n_pad
        layout.append(dict(calls=calls, T=base))

    # window-major desc order: window -> [(r, blk, k, sel_col)]
    nwin = (nchunks + call_cells - 1) // call_cells
    wdescs = [[] for _ in range(nwin)]
    selmap = {}
    n_sel = 0
    for wi in range(nwin):
        for r in range(4):
            call = layout[r]["calls"][wi]
            nblk = call["n_pad"] // P
            offs, k0 = call["offs"], call["k0"]
            for b in range(nblk):
                e0, e1 = b * P, b * P + P
                ks = [k for k in range(call["k0"], call["k1"])
                      if offs[k - k0] < e1 and offs[k - k0 + 1] > e0]
                if not ks:
                    ks = [call["k0"]]
                for k in ks:
                    wdescs[wi].append((r, b, k, n_sel))
                    selmap[(r, wi, b, k)] = n_sel
                    n_sel += 1
        covered = set(d[2] for d in wdescs[wi])
        for k in range(wi * call_cells, min((wi + 1) * call_cells, nchunks)):
            if k not in covered:
                wdescs[wi].append((0, 0, k, n_sel))
                n_sel += 1

    idx16 = [np.zeros((ncr, layout[r]["T"]), np.int16) for r in range(4)]
    rel = np.full((ncr, P, n_sel), -1.0, np.float16)
    wst = [np.zeros((ncr, P, layout[r]["T"] // P), np.float16)
           for r in range(4)]

    for c in range(ncr):
        tr, sl, ww = tab_row[c], slot[c], w[c]
        rr = _range_of(tr)
        ch = sl // P
        o = np.lexsort((sl, ch, rr))
        tr, sl, ww, rr, ch = tr[o], sl[o], ww[o], rr[o], ch[o]
        for r in range(4):
            m = rr == r
            if not m.any():
                continue
            trm, slm, wwm, chm = tr[m], sl[m], ww[m], ch[m]
            cell_cnt = np.zeros(nchunks, np.int64)
            np.add.at(cell_cnt, chm, 1)
            cstart = np.concatenate([[0], np.cumsum(cell_cnt)])
            within = np.arange(len(slm)) - cstart[chm]
            wids = chm // call_cells
            calls = layout[r]["calls"]
            cbase = np.array([cl["base"] for cl in calls], np.int64)
            cell_off = np.zeros(nchunks, np.int64)
            for ci, cl in enumerate(calls):
                for k in range(cl["k0"], cl["k1"]):
                    cell_off[k] = cl["offs"][k - cl["k0"]]
            pos = cbase[wids] + cell_off[chm] + within
            idx16[r][c, pos] = (trm - RLO[r]).astype(np.int16)
            wst[r][c][pos % P, pos // P] = wwm.astype(np.float16)
            relpos = pos - cbase[wids]
            blk = relpos // P
            pp = relpos % P
            cols = np.array([selmap[(r, int(w_), int(b_), int(k_))]
                             for w_, b_, k_ in zip(wids, blk, chm)],
                            np.int64)
            rel[c, pp, cols] = (slm % P).astype(np.float16)
    return dict(layout=layout, wdescs=wdescs, n_sel=n_sel, idx16=idx16,
                rel=rel, wst=wst, nwin=nwin)


def kernel(x, edge_index1, edge_index2, edge_weight1, edge_weight2,
           pos_edge_index, W1, W2, Wlin):
    import concourse.bass as bass
    from concourse import bacc, tile, mybir
    from concourse.bass_utils import run_bass_kernel_spmd
    from concourse import library_config

    f32 = mybir.dt.float32
    f16 = mybir.dt.float16
    i16 = mybir.dt.int16
    eq, mu = mybir.AluOpType.is_equal, mybir.AluOpType.mult
    ACT = mybir.ActivationFunctionType

    x = np.asarray(x, np.float32)
    W1 = np.asarray(W1, np.float32)
    W2 = np.asarray(W2, np.float32)
    Wlin = np.asarray(Wlin, np.float32)
    e1 = np.asarray(edge_index1).astype(np.int64)
    e2 = np.asarray(edge_index2).astype(np.int64)
    w1 = np.asarray(edge_weight1, np.float32)
    w2 = np.asarray(edge_weight2, np.float32)
    pe = np.asarray(pos_edge_index).astype(np.int64)

    # ---------- host index preprocessing ----------
    x_tab = np.zeros((TABROWS, P), np.float16)
    x_tab[:N] = x.astype(np.float16)
    n2row = (np.arange(N) // NSHARD) * SLOTS + (np.arange(N) % NSHARD)

    def shard_by_dest(src_rows, dst, w):
        owner = dst // NSHARD
        ld = dst - owner * NSHARD
        return ([src_rows[owner == c] for c in range(NCORES)],
                [ld[owner == c] for c in range(NCORES)],
                [w[owner == c] for c in range(NCORES)])

    l1 = _prep_stream(*shard_by_dest(e1[0], e1[1], w1), CHUNKS, CALL_CELLS)
    l2 = _prep_stream(*shard_by_dest(n2row[e2[0]], e2[1], w2),
                      CHUNKS, CALL_CELLS)

    # decode: shard pairs by owner core of the endpoint; u from owner(p0),
    # v from owner(p1); host adds the two halves. Per-pair uv rows are
    # dma_gathered from a local DRAM uv table (row = local slot).
    npairs = pe.shape[1]
    own0 = pe[0] // NSHARD
    own1 = pe[1] // NSHARD
    zrow0 = pe[0] % NSHARD
    zrow1 = pe[1] % NSHARD
    u_idx = [zrow0[own0 == c] for c in range(NCORES)]
    v_idx = [zrow1[own1 == c] for c in range(NCORES)]
    maxcnt = max(max(len(a) for a in u_idx), max(len(a) for a in v_idx))
    dcap = ((maxcnt + DWIN - 1) // DWIN) * DWIN
    nwin_d = dcap // DWIN

    def pack_didx(lists):
        arr = np.zeros((NCORES, dcap), np.int16)
        for c in range(NCORES):
            arr[c, :len(lists[c])] = lists[c].astype(np.int16)
        return _wrap_idx(arr)

    u_widx = pack_didx(u_idx)
    v_widx = pack_didx(v_idx)

    idx_arr = {}
    for key, pr in (("l1", l1), ("l2", l2)):
        for r in range(4):
            idx_arr[(key, r)] = _wrap_idx(pr["idx16"][r])

    # wu = W2 @ [WlinA.T | WlinB.T]  [128, 4]
    wcat = np.concatenate([Wlin[:, :P].T, Wlin[:, P:].T], axis=1)  # [128,4]
    wu = (W2 @ wcat).astype(np.float16)                            # [128,4]

    iota_np = np.broadcast_to(
        np.arange(P, dtype=np.float16)[None, :], (P, P)).copy()

    # ---------- device program ----------
    nc = bacc.Bacc("TRN2", target_bir_lowering=False, debug=False,
                   num_devices=NCORES, num_swdge_queues=4)

    def din(name, shape, dt=f16):
        return nc.dram_tensor(name, list(shape), dt, kind="ExternalInput").ap()

    xt = din("x_tab", (TABROWS, P))
    w1t = din("W1r", (P, P))
    wu_in = din("wu4", (P, 4))
    iota_in = din("iota", (P, P))
    idx_in = {k: din(f"idx_{k[0]}_{k[1]}", v.shape[1:], i16)
              for k, v in idx_arr.items()}
    rel_in = {key: din(f"rel_{key}", (P, pr["n_sel"]))
              for key, pr in (("l1", l1), ("l2", l2))}
    wst_in = {(key, r): din(f"wst_{key}_{r}", (P, pr["layout"][r]["T"] // P))
              for key, pr in (("l1", l1), ("l2", l2)) for r in range(4)}
    uidx_in = din("u_idx", (P, dcap // 16), i16)
    vidx_in = din("v_idx", (P, dcap // 16), i16)

    u_out = nc.dram_tensor("u_out", [P, (dcap // P) * 4], f16,
                           kind="ExternalOutput").ap()
    v_out = nc.dram_tensor("v_out", [P, (dcap // P) * 4], f16,
                           kind="ExternalOutput").ap()
    h_slice = nc.dram_tensor("h_slice", [SLOTS, P], f16)
    h_tab = nc.dram_tensor("h_tab", [TABROWS, P], f16, addr_space="Shared")
    uv_d = nc.dram_tensor("uv_d", [SLOTS, P], f16)

    qn = [0]

    def next_q():
        qn[0] = (qn[0] + 1) % 4
        return qn[0]

    with tile.TileContext(nc) as tc:
        with (
            tc.tile_pool(name="meta", bufs=1) as mp,
            tc.tile_pool(name="stage", bufs=2) as sgp,
            tc.tile_pool(name="idxp", bufs=1) as ixp,
            tc.tile_pool(name="selp", bufs=4) as selp,
            tc.tile_pool(name="work", bufs=4) as wp,
            tc.tile_pool(name="dago", bufs=2) as dgp,
            tc.tile_pool(name="psA", bufs=6, space="PSUM") as ppA,
            tc.tile_pool(name="psB", bufs=2, space="PSUM") as ppB,
        ):
            nc.gpsimd.load_library(library_config.mlp)
            iota_t = mp.tile([P, P], f16, name="iota_t")
            nc.sync.dma_start(iota_t[:], iota_in[:])
            w1_sb = mp.tile([P, P], f16, name="w1_sb")
            nc.sync.dma_start(w1_sb[:], w1t[:])
            wu_sb = mp.tile([P, 4], f16, name="wu_sb")
            nc.sync.dma_start(wu_sb[:], wu_in[:])
            # uv accumulator: chunk k at cols [4k, 4k+4)
            uv_sb = mp.tile([P, 4 * CHUNKS], f16, name="uv_sb")

            def run_layer(key, pr, tab_ap, consume):
                rel_sb = ixp.tile([P, pr["n_sel"]], f16,
                                  name=f"rel_{key}", tag="relt")
                nc.sync.dma_start(rel_sb[:], rel_in[key][:])
                idx_sb = []
                wst_sb = []
                for r in range(4):
                    cols = pr["layout"][r]["T"] // 16
                    it = ixp.tile([P, cols], i16, name=f"ix_{key}_{r}",
                                  tag=f"ix{r}")
                    nc.sync.dma_start(it[:], idx_in[(key, r)][:])
                    idx_sb.append(it)
                    wt = ixp.tile([P, pr["layout"][r]["T"] // P], f16,
                                  name=f"wst_{key}_{r}", tag=f"wst{r}")
                    nc.sync.dma_start(wt[:], wst_in[(key, r)][:])
                    wst_sb.append(wt)

                for wi in range(pr["nwin"]):
                    # gather + weight-scale the window's 4 range calls
                    st_t = {}
                    for r in range(4):
                        call = pr["layout"][r]["calls"][wi]
                        npad = call["n_pad"]
                        nblk = npad // P
                        st = sgp.tile([P, nblk * P], f16,
                                      name=f"st_{key}_{wi}_{r}",
                                      tag=f"stage{r}")
                        nc.gpsimd.dma_gather(
                            st[:].rearrange("p (c e) -> p c e", e=P),
                            tab_ap[RLO[r]:],
                            idx_sb[r][:, call["base"] // 16:
                                      (call["base"] + npad) // 16],
                            npad, npad, P,
                            queue_num=next_q(), single_packet=False)
                        b0 = call["base"] // P
                        w_exp = (wst_sb[r][:, b0:b0 + nblk].unsqueeze(2)
                                 .to_broadcast([P, nblk, P]))
                        nc.vector.tensor_tensor(
                            out=st[:].rearrange("p (b c) -> p b c", c=P),
                            in0=st[:].rearrange("p (b c) -> p b c", c=P),
                            in1=w_exp, op=mu)
                        st_t[r] = st

                    descs = pr["wdescs"][wi]
                    # psum banks for this window: 4 chunks per [128,512] bank
                    k0 = wi * CALL_CELLS
                    kset = sorted(set(d[2] for d in descs))
                    nbank = (CALL_CELLS + 3) // 4
                    banks = [ppA.tile([P, 4 * P], f32, space="PSUM",
                                      name=f"pt_{key}_{wi}_{t}", tag="agg")
                             for t in range(nbank)]

                    def pslice(k):
                        d = k - k0
                        return banks[d // 4][:, (d % 4) * P:(d % 4) * P + P]

                    # start=True resets the ENTIRE psum bank, so scope the
                    # start/stop flags per bank, not per chunk.
                    first = {}
                    last = {}
                    for j, (r, b, k, s) in enumerate(descs):
                        bk = (k - k0) // 4
                        first.setdefault(bk, j)
                        last[bk] = j
                    # wide sel builds in batches
                    sel_t = {}
                    for j0 in range(0, len(descs), SELBATCH):
                        nb = min(SELBATCH, len(descs) - j0)
                        s0 = descs[j0][3]
                        selt = selp.tile([P, nb * P], f16,
                                         name=f"sel_{key}_{wi}_{j0}",
                                         tag="sel")
                        nc.vector.tensor_tensor(
                            out=selt[:].rearrange("p (b c) -> p b c", c=P),
                            in0=iota_t[:].unsqueeze(1).to_broadcast(
                                [P, nb, P]),
                            in1=rel_sb[:, s0:s0 + nb].unsqueeze(2)
                                .to_broadcast([P, nb, P]),
                            op=eq)
                        for jj in range(nb):
                            sel_t[j0 + jj] = (selt, jj)
                    for j, (r, b, k, s) in enumerate(descs):
                        selt, jj = sel_t[j]
                        bk = (k - k0) // 4
                        nc.tensor.matmul(
                            pslice(k),
                            lhsT=st_t[r][:, b * P:(b + 1) * P],
                            rhs=selt[:, jj * P:(jj + 1) * P],
                            start=(first[bk] == j), stop=(last[bk] == j),
                            skip_group_check=True)
                    for k in kset:
                        consume(k, pslice(k))

            def consume_l1(k, psum_t):
                at_sb = wp.tile([P, P], f16, name=f"a1_{k}", tag="at")
                nc.scalar.activation(at_sb[:], psum_t[:], ACT.Copy)
                h_ps = ppB.tile([P, P], f32, space="PSUM",
                                name=f"h1_{k}", tag="psB")
                nc.tensor.matmul(h_ps[:], lhsT=at_sb[:], rhs=w1_sb[:],
                                 start=True, stop=True)
                h_sb = wp.tile([P, P], f16, name=f"h1s_{k}", tag="h")
                nc.scalar.activation(h_sb[:], h_ps[:], ACT.Relu)
                nc.sync.dma_start(h_slice[k * P:(k + 1) * P, :], h_sb[:])

            def consume_l2(k, psum_t):
                at_sb = wp.tile([P, P], f16, name=f"a2_{k}", tag="at")
                nc.scalar.activation(at_sb[:], psum_t[:], ACT.Copy)
                uv_ps = ppB.tile([P, 4], f32, space="PSUM",
                                 name=f"uv_{k}", tag="psB")
                nc.tensor.matmul(uv_ps[:], lhsT=at_sb[:], rhs=wu_sb[:],
                                 start=True, stop=True)
                nc.scalar.activation(uv_sb[:, 4 * k:4 * k + 4], uv_ps[:],
                                     ACT.Copy)

            run_layer("l1", l1, xt, consume_l1)
            nc.gpsimd.collective_compute(
                "AllGather", mybir.AluOpType.bypass,
                replica_groups=[list(range(NCORES))],
                ins=[h_slice[:]], outs=[h_tab[:]])
            run_layer("l2", l2, h_tab[:], consume_l2)
            # uv_sb [p, 4k+c] -> uv_d row k*128+p, col c (one strided DMA)
            nc.sync.dma_start(
                uv_d[:, 0:4].rearrange("(k p) c -> p k c", p=P),
                uv_sb[:].rearrange("p (k c) -> p k c", c=4))

            # ---------- decode: gather uv rows per pair ----------
            uix = ixp.tile([P, dcap // 16], i16, name="uix", tag="dix")
            nc.sync.dma_start(uix[:], uidx_in[:])
            vix = ixp.tile([P, dcap // 16], i16, name="vix", tag="dix2")
            nc.sync.dma_start(vix[:], vidx_in[:])
            nblk_d = DWIN // P
            for name, ixt, outd in (("u", uix, u_out), ("v", vix, v_out)):
                for wi in range(nwin_d):
                    ag = dgp.tile([P, DWIN], f16, name=f"ag_{name}_{wi}",
                                  tag="ag")
                    nc.gpsimd.dma_gather(
                        ag[:].rearrange("p (c e) -> p c e", e=P),
                        uv_d[:],
                        ixt[:, wi * (DWIN // 16):(wi + 1) * (DWIN // 16)],
                        DWIN, DWIN, P,
                        queue_num=next_q(), single_packet=False)
                    uc = dgp.tile([P, nblk_d * 4], f16,
                                  name=f"uc_{name}_{wi}", tag="uc")
                    nc.vector.tensor_copy(
                        uc[:].rearrange("p (b c) -> p b c", c=4),
                        ag[:].rearrange("p (b e) -> p b e", e=P)[:, :, 0:4])
                    nc.sync.dma_start(
                        outd[:, wi * nblk_d * 4:(wi + 1) * nblk_d * 4],
                        uc[:])

    nc.compile()

    # ---------- stage inputs & run ----------
    in_maps = []
    for c in range(NCORES):
        m = {"x_tab": x_tab, "W1r": W1.astype(np.float16),
             "wu4": wu, "iota": iota_np,
             "u_idx": u_widx[c], "v_idx": v_widx[c]}
        for key, pr in (("l1", l1), ("l2", l2)):
            m[f"rel_{key}"] = np.ascontiguousarray(pr["rel"][c])
            for r in range(4):
                m[f"idx_{key}_{r}"] = idx_arr[(key, r)][c]
                m[f"wst_{key}_{r}"] = np.ascontiguousarray(pr["wst"][r][c])
        in_maps.append(m)

    res = run_bass_kernel_spmd(nc, in_maps, core_ids=list(range(NCORES)),
                               trace=globals().get("TRACE", False))
    globals()["LAST_EXEC_NS"] = res.exec_time_ns
    globals()["LAST_RES"] = res

    # u_out[p, g*4+c] holds comp c for the pair at position g*128+p of the
    # core's stream; reshape to [dcap, 4] in stream order.
    def unpack(key):
        a = np.stack([res.results[c][key] for c in range(NCORES)])
        a = a.reshape(NCORES, P, dcap // P, 4).transpose(0, 2, 1, 3)
        return a.reshape(NCORES, dcap, 4).astype(np.float32)

    u_res = unpack("u_out")
    v_res = unpack("v_out")
    pos0 = np.zeros(npairs, np.int64)
    pos1 = np.zeros(npairs, np.int64)
    cnt0 = np.zeros(NCORES, np.int64)
    cnt1 = np.zeros(NCORES, np.int64)
    for p in range(npairs):
        pos0[p] = cnt0[own0[p]]; cnt0[own0[p]] += 1
        pos1[p] = cnt1[own1[p]]; cnt1[own1[p]] += 1
    out = u_res[own0, pos0, 0:2] + v_res[own1, pos1, 2:4]
    return np.ascontiguousarray(out, np.float32)
